# revision 4
# baseline (speedup 1.0000x reference)
"""Trainium2 Bass kernel for nn_BilinearFusion.

out[b] = sum_h [ x1_h(b)·W1_h + b1_h + x2_h(b)·W2_h + x2_h(b)^T W3_h x1_h(b) ]

Math used here:
  t3[b]   = sum_h sum_o x2[b, h, o] * Y[b, h, o],  Y[b,h,o] = sum_i x1[b,h,i] W3[h,o,i]
  t1 + t2 = x1[b,:]·w1_flat + x2[b,:]·w2_flat      (w*_flat = per-head weights concatenated)
  c       = sum_h b1[h]

Per-core plan (data-parallel over batch, 8 cores x 2048 rows, 16 tiles of 128):
  - cast x1/x2 tiles to bf16 into one combined tile xc = [x1b | x2b]
  - per head: PE-transpose x1b_h -> xt (PSUM), ACT-copy to SBUF bf16,
    matmul Y_h = xt^T W3T_h (bf16, fp32 PSUM), ACT-copy Y to SBUF bf16
  - DVE tensor_tensor_reduce #1: sum(Y * x2b) + c        (accumulates t3 + c)
  - DVE tensor_tensor_reduce #2: sum(xc * [w1|w2]_bcast) (accumulates t1 + t2)
  - DMA the [128,1] result column per tile
"""

import numpy as np
import ml_dtypes

import concourse.bass as bass
import concourse.tile as tile
from concourse import bacc, mybir
from concourse.bass_utils import run_bass_kernel_spmd

BF16 = ml_dtypes.bfloat16

B, D, HEAD, DIM = 16384, 1024, 8, 128
NCORES = 8
ROWS = B // NCORES          # 2048 rows per core
P = 128                     # partition tile (rows per tile)
NT = ROWS // P              # 16 tiles per core

_nc_cache = {}


def build_nc(c_b1: float):
    nc = bacc.Bacc(target_bir_lowering=False)
    f32 = mybir.dt.float32
    bf16 = mybir.dt.bfloat16

    x1_d = nc.dram_tensor("x1", [ROWS, D], f32, kind="ExternalInput")
    x2_d = nc.dram_tensor("x2", [ROWS, D], f32, kind="ExternalInput")
    w3t_d = nc.dram_tensor("w3t", [DIM, HEAD, DIM], bf16, kind="ExternalInput")
    w12_d = nc.dram_tensor("w12", [1, 2 * D], bf16, kind="ExternalInput")
    ident_d = nc.dram_tensor("ident", [P, P], bf16, kind="ExternalInput")
    out_d = nc.dram_tensor("out", [NT, P], f32, kind="ExternalOutput")

    with tile.TileContext(nc) as tc:
        with (
            tc.tile_pool(name="const", bufs=1) as const_pool,
            tc.tile_pool(name="ins", bufs=3) as in_pool,
            tc.tile_pool(name="xc", bufs=3) as xc_pool,
            tc.tile_pool(name="xt", bufs=3) as xt_pool,
            tc.tile_pool(name="ysb", bufs=2) as ysb_pool,
            tc.tile_pool(name="scr", bufs=2) as scr_pool,
            tc.tile_pool(name="acc", bufs=4) as acc_pool,
            tc.tile_pool(name="xtps", bufs=2, space="PSUM") as xtps_pool,
            tc.tile_pool(name="yps", bufs=4, space="PSUM") as yps_pool,
        ):
            # One-time constants
            w3t_sb = const_pool.tile([DIM, HEAD, DIM], bf16)
            nc.sync.dma_start(out=w3t_sb, in_=w3t_d[:])
            ident = const_pool.tile([P, P], bf16)
            nc.sync.dma_start(out=ident, in_=ident_d[:])
            w12bc = const_pool.tile([P, 2 * D], bf16)
            nc.sync.dma_start(out=w12bc, in_=w12_d[:].to_broadcast([P, 2 * D]))

            for t in range(NT):
                rs = t * P
                x1_t = in_pool.tile([P, D], f32, tag="x1t")
                nc.sync.dma_start(out=x1_t, in_=x1_d[rs:rs + P, :])
                x2_t = in_pool.tile([P, D], f32, tag="x2t")
                nc.sync.dma_start(out=x2_t, in_=x2_d[rs:rs + P, :])

                xc = xc_pool.tile([P, 2 * D], bf16)
                # cast fp32 -> bf16; x1 half on gpsimd, x2 half on ACT
                nc.gpsimd.tensor_copy(xc[:, 0:D], x1_t)
                nc.scalar.copy(out=xc[:, D:2 * D], in_=x2_t)

                ysb = ysb_pool.tile([P, 2, 4, DIM], bf16)
                for g in range(2):  # head groups of 4
                    xt_ps = xtps_pool.tile([P, 4, DIM], bf16)
                    for j in range(4):
                        h = 4 * g + j
                        nc.tensor.transpose(
                            xt_ps[:, j, :], xc[:, h * DIM:(h + 1) * DIM], ident
                        )
                    xt_sb = xt_pool.tile([P, 4, DIM], bf16)
                    nc.scalar.copy(out=xt_sb, in_=xt_ps)

                    y_ps = yps_pool.tile([P, 4, DIM], f32)
                    for j in range(4):
                        h = 4 * g + j
                        nc.tensor.matmul(
                            y_ps[:, j, :],
                            xt_sb[:, j, :],        # lhsT (stationary) = x1_h^T
                            w3t_sb[:, h, :],       # rhs (moving) = W3_h^T
                            start=True, stop=True,
                        )
                    nc.scalar.copy(out=ysb[:, g, :, :], in_=y_ps)

                # DVE: prod = [Y*x2b | xc*w12bc], one reduce, add bias-sum
                prod = scr_pool.tile([P, 3 * D], bf16, tag="prod")
                nc.vector.tensor_mul(
                    prod[:, 0:D].rearrange("p (a b c) -> p a b c", a=2, b=4),
                    ysb,
                    xc[:, D:2 * D].rearrange("p (a b c) -> p a b c", a=2, b=4),
                )
                nc.vector.tensor_mul(prod[:, D:3 * D], xc, w12bc)
                acc0 = acc_pool.tile([P, 1], f32, tag="a0")
                nc.vector.tensor_reduce(
                    acc0, prod, axis=mybir.AxisListType.X,
                    op=mybir.AluOpType.add,
                )
                acc1 = acc_pool.tile([P, 1], f32, tag="a1")
                nc.vector.tensor_scalar_add(acc1, acc0, c_b1)
                nc.sync.dma_start(out=out_d[t, :], in_=acc1)

    nc.finalize()
    return nc


def kernel(x1, x2, W1, b1, W2, W3):
    c_b1 = float(np.asarray(b1, dtype=np.float64).sum())
    key = round(c_b1, 12)
    if key not in _nc_cache:
        _nc_cache[key] = build_nc(c_b1)
    nc = _nc_cache[key]

    # W3 is [h, o, i]; matmul rhs needs [i (partitions), h, o]
    w3t = np.ascontiguousarray(np.transpose(np.asarray(W3), (2, 0, 1))).astype(BF16)
    w12 = np.concatenate(
        [np.asarray(W1).reshape(-1), np.asarray(W2).reshape(-1)]
    ).astype(BF16).reshape(1, 2 * D)
    ident = np.eye(P, dtype=BF16)

    x1 = np.ascontiguousarray(np.asarray(x1, dtype=np.float32))
    x2 = np.ascontiguousarray(np.asarray(x2, dtype=np.float32))

    in_maps = []
    for c in range(NCORES):
        sl = slice(c * ROWS, (c + 1) * ROWS)
        in_maps.append({
            "x1": x1[sl], "x2": x2[sl],
            "w3t": w3t, "w12": w12, "ident": ident,
        })

    res = run_bass_kernel_spmd(nc, in_maps, core_ids=list(range(NCORES)))
    out = np.concatenate([res.results[c]["out"].reshape(-1) for c in range(NCORES)])
    return out
